# revision 2
# baseline (speedup 1.0000x reference)
"""Trainium2 Bass kernel for nn_CenterLossLayer.

Data-parallel over the batch dim across 8 NeuronCores:
  - each core processes 8192 rows of features/onehot; centers replicated
  - per 128-row tile:
      selected = onehot @ centers      (PE: transpose onehot, then matmul)
      diff     = selected - features   (DVE, bf16 out)
      result   = rowsum(diff^2)        (ACT square + accum)
      delta   += onehot.T @ diff       (PE, PSUM accumulation over tiles)
      counts  += onehot.T @ ones       (PE)
  - [delta | counts] AllReduced across the 8 cores (device-side psum)
  - new_centers = centers - ALPHA * delta_tot / (counts_tot + 1) on-chip

kernel(**inputs) takes the FULL unsharded inputs and returns the full
(result [65536,1], new_centers [100,512]) like the reference.
"""

import numpy as np

ALPHA = 0.5
C = 100        # num classes
D = 512        # feature dim
B = 65536      # global batch
N_CORES = 8
B_LOCAL = B // N_CORES    # 8192
P = 128
N_TILES = B_LOCAL // P    # 64
CC_W = D + 1              # AllReduce payload width: [delta | counts]

_NC_CACHE = None


def _build(n_tiles=N_TILES, n_cores=N_CORES, with_cc=True):
    import concourse.tile as tile
    from concourse import bacc, mybir
    from concourse.masks import make_identity

    f32 = mybir.dt.float32
    bf16 = mybir.dt.bfloat16
    sub = mybir.AluOpType.subtract
    mult = mybir.AluOpType.mult

    b_local = n_tiles * P
    nc = bacc.Bacc("TRN2", target_bir_lowering=False, debug=False,
                   num_devices=n_cores)
    feat_d = nc.dram_tensor("features", [b_local, D], f32,
                            kind="ExternalInput").ap()
    oh_d = nc.dram_tensor("onehot", [b_local, C], f32,
                          kind="ExternalInput").ap()
    cen_d = nc.dram_tensor("centers", [C, D], f32, kind="ExternalInput").ap()
    # result in (p, t) layout: row b = t*128 + p lives at [p, t]
    res_d = nc.dram_tensor("result_pt", [P, n_tiles], f32,
                           kind="ExternalOutput").ap()
    ncen_d = nc.dram_tensor("new_centers", [C, D], f32,
                            kind="ExternalOutput").ap()

    feat_t = feat_d.rearrange("(t p) d -> t p d", p=P)
    oh_td = oh_d.rearrange("(t p) c -> t p c", p=P)

    with tile.TileContext(nc) as tc:
        with (
            tc.tile_pool(name="const", bufs=1) as cpool,
            tc.tile_pool(name="feat", bufs=4) as fpool,
            tc.tile_pool(name="oh", bufs=4) as opool,
            tc.tile_pool(name="ohbf", bufs=4) as obfpool,
            tc.tile_pool(name="ohT", bufs=3) as tpool,
            tc.tile_pool(name="diff", bufs=3) as dpool,
            tc.tile_pool(name="sq", bufs=2) as sqpool,
            tc.tile_pool(name="fin", bufs=1) as finpool,
            tc.tile_pool(name="ps_sel", bufs=2, space="PSUM") as ps_sel,
            tc.tile_pool(name="ps_tr", bufs=2, space="PSUM") as ps_tr,
            tc.tile_pool(name="ps_acc", bufs=1, space="PSUM") as ps_acc,
            tc.tile_pool(name="dram", bufs=1, space="DRAM") as drampool,
        ):
            # ---- constants ----
            ident = cpool.tile([P, P], bf16)
            make_identity(nc, ident)
            ones_bf = cpool.tile([P, 1], bf16)
            nc.vector.memset(ones_bf, 1.0)
            cen_sb = cpool.tile([P, D], f32)
            nc.sync.dma_start(cen_sb[:C, :], cen_d)
            cen_bf = cpool.tile([P, D], bf16)
            nc.vector.tensor_copy(cen_bf[:C, :], cen_sb[:C, :])

            res_sb = finpool.tile([P, n_tiles], f32)
            psum_delta = ps_acc.tile([P, D], f32)
            psum_cnt = ps_acc.tile([P, 1], f32)

            prev_diff = None
            for t in range(n_tiles):
                f_t = fpool.tile([P, D], f32)
                nc.sync.dma_start(f_t, feat_t[t])
                oh_f = opool.tile([P, C], f32)
                nc.sync.dma_start(oh_f, oh_td[t])
                oh_bf = obfpool.tile([P, C], bf16)
                nc.vector.tensor_copy(oh_bf, oh_f)

                # onehot tile transposed: [C, P] via PE transpose
                tr = ps_tr.tile([P, P], bf16)
                nc.tensor.transpose(tr[:C, :], oh_bf, ident)
                ohT = tpool.tile([P, P], bf16)
                nc.scalar.copy(ohT[:C, :], tr[:C, :])

                # selected = onehot @ centers  -> PSUM [P, D] fp32
                sel = ps_sel.tile([P, D], f32)
                nc.tensor.matmul(sel, ohT[:C, :], cen_bf[:C, :],
                                 start=True, stop=True)

                # software-pipelined accumulating matmuls for tile t-1
                if prev_diff is not None:
                    pd, poh = prev_diff
                    nc.tensor.matmul(psum_delta[:C, :], poh, pd,
                                     start=(t == 1), stop=False,
                                     skip_group_check=True)
                    nc.tensor.matmul(psum_cnt[:C, :], poh, ones_bf,
                                     start=(t == 1), stop=False,
                                     skip_group_check=True)

                # diff = selected - features (bf16 out)
                diff = dpool.tile([P, D], bf16)
                nc.vector.tensor_tensor(out=diff, in0=sel, in1=f_t, op=sub)

                # result[:, t] = rowsum(diff^2)
                sq = sqpool.tile([P, D], f32)
                nc.scalar.activation(sq, diff,
                                     mybir.ActivationFunctionType.Square,
                                     accum_out=res_sb[:, t:t + 1])
                prev_diff = (diff, oh_bf)

            pd, poh = prev_diff
            nc.tensor.matmul(psum_delta[:C, :], poh, pd,
                             start=(n_tiles == 1), stop=True,
                             skip_group_check=True)
            nc.tensor.matmul(psum_cnt[:C, :], poh, ones_bf,
                             start=(n_tiles == 1), stop=True,
                             skip_group_check=True)

            # ---- AllReduce [delta | counts] across the 8 cores ----
            cat = finpool.tile([P, CC_W], f32)
            nc.scalar.copy(cat[:C, :D], psum_delta[:C, :])
            nc.scalar.copy(cat[:C, D:D + 1], psum_cnt[:C, :])
            if with_cc:
                cc_in = drampool.tile([C, CC_W], f32)
                cc_out = drampool.tile([C, CC_W], f32, addr_space="Shared")
                nc.sync.dma_start(cc_in, cat[:C, :])
                nc.gpsimd.collective_compute(
                    "AllReduce", mybir.AluOpType.add,
                    replica_groups=[list(range(n_cores))],
                    ins=[cc_in.opt()], outs=[cc_out.opt()],
                )
                tot = finpool.tile([P, CC_W], f32)
                nc.sync.dma_start(tot[:C, :], cc_out)
            else:
                tot = cat

            # ---- new_centers = centers - ALPHA * delta / (counts + 1) ----
            cntp1 = finpool.tile([P, 1], f32)
            nc.scalar.add(cntp1[:C, :], tot[:C, D:D + 1], 1.0)
            recip = finpool.tile([P, 1], f32)
            nc.vector.reciprocal(recip[:C, :], cntp1[:C, :])
            scaled = finpool.tile([P, D], f32)
            nc.vector.tensor_scalar(out=scaled[:C, :], in0=tot[:C, :D],
                                    scalar1=recip[:C, :], scalar2=ALPHA,
                                    op0=mult, op1=mult)
            ncen_sb = finpool.tile([P, D], f32)
            nc.vector.tensor_tensor(out=ncen_sb[:C, :], in0=cen_sb[:C, :],
                                    in1=scaled[:C, :], op=sub)

            nc.sync.dma_start(ncen_d, ncen_sb[:C, :])
            nc.sync.dma_start(res_d, res_sb)

    nc.compile()
    return nc


def _get_nc():
    global _NC_CACHE
    if _NC_CACHE is None:
        _NC_CACHE = _build()
    return _NC_CACHE


def _run(inputs, trace=False, **run_kwargs):
    from concourse.bass_utils import run_bass_kernel_spmd

    feats = np.ascontiguousarray(np.asarray(inputs["features"],
                                            dtype=np.float32))
    onehot = np.ascontiguousarray(np.asarray(inputs["onehot"],
                                             dtype=np.float32))
    centers = np.ascontiguousarray(np.asarray(inputs["centers"],
                                              dtype=np.float32))
    assert feats.shape == (B, D) and onehot.shape == (B, C)
    assert centers.shape == (C, D)

    nc = _get_nc()
    in_maps = []
    for k in range(N_CORES):
        sl = slice(k * B_LOCAL, (k + 1) * B_LOCAL)
        in_maps.append({
            "features": feats[sl],
            "onehot": onehot[sl],
            "centers": centers,
        })
    bres = run_bass_kernel_spmd(nc, in_maps, core_ids=list(range(N_CORES)),
                                trace=trace, **run_kwargs)
    outs = bres.results
    result = np.concatenate(
        [outs[k]["result_pt"].T.reshape(-1, 1) for k in range(N_CORES)],
        axis=0,
    ).astype(np.float32)
    new_centers = outs[0]["new_centers"].astype(np.float32)
    return (result, new_centers), bres


def kernel(**inputs):
    (result, new_centers), _ = _run(inputs, trace=False)
    return result, new_centers


# revision 6
# speedup vs baseline: 1.1778x; 1.1778x over previous
"""Trainium2 Bass kernel for nn_CenterLossLayer.

Data-parallel over the batch dim across 8 NeuronCores:
  - each core processes 8192 rows of features/onehot; centers replicated
  - per 4-tile iteration (tile = 128 batch rows):
      fused DMA loads (features [128,2048], onehot [128,400])
      onehot cast to bf16 on GpSimd
      4x PE transpose of onehot tiles into one PSUM tile, one DVE copy out
      selected = onehot @ centers      (PE matmul per tile, 2-tile PSUM groups)
      diff     = selected - features   (DVE, [128,1024] per op, bf16 out)
      result   = rowsum(diff^2)        (ACT square + accum per tile)
      delta   += onehot.T @ diff       (PE, PSUM accumulation over all tiles)
      countsT += ones.T @ onehot       (PE, [1,400] PSUM row accumulation)
  - [delta | counts] ReduceScattered across the 8 cores (device-side psum);
    each core finishes its 13-class shard of new_centers from a host-sharded
    copy of centers.

kernel(**inputs) takes the FULL unsharded inputs and returns the full
(result [65536,1], new_centers [100,512]) like the reference.
"""

import numpy as np

ALPHA = 0.5
C = 100        # num classes
C_PAD = 104    # padded to 8*13 for ReduceScatter
C_SH = C_PAD // 8
D = 512        # feature dim
B = 65536      # global batch
N_CORES = 8
B_LOCAL = B // N_CORES    # 8192
P = 128
N_TILES = B_LOCAL // P    # 64
TPI = 4                   # tiles per iteration
CC_W = D + 1              # RS payload width: [delta | counts]

_NC_CACHE = None


def _build(n_tiles=N_TILES, n_cores=N_CORES, with_cc=True):
    import concourse.tile as tile
    from concourse import bacc, mybir
    from concourse.masks import make_identity

    f32 = mybir.dt.float32
    bf16 = mybir.dt.bfloat16
    sub = mybir.AluOpType.subtract
    mult = mybir.AluOpType.mult

    assert n_tiles % TPI == 0
    n_iters = n_tiles // TPI
    b_local = n_tiles * P

    nc = bacc.Bacc("TRN2", target_bir_lowering=False, debug=False,
                   num_devices=n_cores)
    feat_d = nc.dram_tensor("features", [b_local, D], f32,
                            kind="ExternalInput").ap()
    oh_d = nc.dram_tensor("onehot", [b_local, C], f32,
                          kind="ExternalInput").ap()
    cen_d = nc.dram_tensor("centers", [C, D], f32, kind="ExternalInput").ap()
    censh_d = nc.dram_tensor("centers_shard", [C_SH, D], f32,
                             kind="ExternalInput").ap()
    # result in (p, t) layout: row b = t*128 + p lives at [p, t]
    res_d = nc.dram_tensor("result_pt", [P, n_tiles], f32,
                           kind="ExternalOutput").ap()
    ncen_d = nc.dram_tensor("new_centers_shard", [C_SH, D], f32,
                            kind="ExternalOutput").ap()

    # iteration views: (i, p, j, x) with j = tile within iteration
    feat_v = feat_d.rearrange("(i j p) d -> i p j d", p=P, j=TPI)
    oh_v = oh_d.rearrange("(i j p) c -> i p j c", p=P, j=TPI)

    with tile.TileContext(nc) as tc:
        with (
            tc.tile_pool(name="const", bufs=1) as cpool,
            tc.tile_pool(name="feat", bufs=3) as fpool,
            tc.tile_pool(name="oh", bufs=3) as opool,
            tc.tile_pool(name="ohbf", bufs=3) as obfpool,
            tc.tile_pool(name="ohT", bufs=3) as tpool,
            tc.tile_pool(name="diff", bufs=4) as dpool,
            tc.tile_pool(name="sq", bufs=3) as sqpool,
            tc.tile_pool(name="fin", bufs=1) as finpool,
            tc.tile_pool(name="ps_sel", bufs=2, space="PSUM") as ps_sel,
            tc.tile_pool(name="ps_tr", bufs=2, space="PSUM") as ps_tr,
            tc.tile_pool(name="ps_acc", bufs=1, space="PSUM") as ps_acc,
            tc.tile_pool(name="dram", bufs=1, space="DRAM") as drampool,
        ):
            # ---- constants ----
            ident = cpool.tile([P, P], bf16)
            make_identity(nc, ident)
            one_f32 = cpool.tile([P, 1], f32)
            nc.vector.memset(one_f32, 1.0)
            ones_bf = cpool.tile([P, 1], bf16)
            nc.vector.memset(ones_bf, 1.0)
            cen_sb = cpool.tile([P, D], f32)
            nc.sync.dma_start(cen_sb[:C, :], cen_d)
            censh_sb = cpool.tile([C_SH, D], f32)
            nc.sync.dma_start(censh_sb, censh_d)
            cen_bf = cpool.tile([P, D], bf16)
            nc.vector.tensor_copy(cen_bf[:C, :], cen_sb[:C, :])

            res_sb = finpool.tile([P, n_tiles], f32)
            psum_delta = ps_acc.tile([P, D], f32)
            psum_cntT = ps_acc.tile([1, TPI * C], f32)

            for i in range(n_iters):
                f_t = fpool.tile([P, TPI * D], f32)
                nc.sync.dma_start(f_t.rearrange("p (j d) -> p j d", j=TPI),
                                  feat_v[i])
                oh_f = opool.tile([P, TPI * C], f32)
                nc.sync.dma_start(oh_f.rearrange("p (j c) -> p j c", j=TPI),
                                  oh_v[i])
                oh_bf = obfpool.tile([P, TPI * C], bf16)
                nc.gpsimd.tensor_copy(oh_bf, oh_f)

                # 4 transposed onehot tiles -> one PSUM tile -> one SBUF copy
                tr4 = ps_tr.tile([P, TPI * P], bf16)
                for j in range(TPI):
                    nc.tensor.transpose(
                        tr4[:C, j * P:(j + 1) * P],
                        oh_bf[:, j * C:(j + 1) * C], ident)
                ohT = tpool.tile([P, TPI * P], bf16)
                nc.vector.tensor_copy(ohT[:C, :], tr4[:C, :])

                # countsT row accumulation: [1, TPI*C] += ones.T @ onehot
                nc.tensor.matmul(psum_cntT[:, :], ones_bf, oh_bf,
                                 start=(i == 0), stop=(i == n_iters - 1),
                                 skip_group_check=True)

                for h in range(TPI // 2):   # 2-tile selected/diff groups
                    sel2 = ps_sel.tile([P, 2 * D], f32)
                    for g in range(2):
                        j = h * 2 + g
                        nc.tensor.matmul(
                            sel2[:, g * D:(g + 1) * D],
                            ohT[:C, j * P:(j + 1) * P], cen_bf[:C, :],
                            start=True, stop=True, skip_group_check=True)
                    diff2 = dpool.tile([P, 2 * D], bf16)
                    nc.vector.tensor_tensor(
                        out=diff2, in0=sel2,
                        in1=f_t[:, h * 2 * D:(h + 1) * 2 * D], op=sub)
                    for g in range(2):
                        j = h * 2 + g
                        t = i * TPI + j
                        sq = sqpool.tile([P, D], bf16)
                        nc.scalar.activation(
                            sq, diff2[:, g * D:(g + 1) * D],
                            mybir.ActivationFunctionType.Square,
                            accum_out=res_sb[:, t:t + 1])
                        nc.tensor.matmul(
                            psum_delta[:C, :],
                            oh_bf[:, j * C:(j + 1) * C],
                            diff2[:, g * D:(g + 1) * D],
                            start=(t == 0), stop=(t == n_tiles - 1),
                            skip_group_check=True)

            # ---- counts row -> column via tiny PE transpose ----
            cntT_sb = finpool.tile([1, TPI * C], f32)
            nc.vector.tensor_copy(cntT_sb, psum_cntT)
            # sum the TPI sub-rows: [1, (j c)] -> [1, c] (reduce over j)
            cnt_row = finpool.tile([1, C], f32)
            nc.vector.reduce_sum(
                cnt_row,
                cntT_sb.rearrange("one (j c) -> one c j", j=TPI),
                axis=mybir.AxisListType.X)
            cnt_ps = ps_sel.tile([P, 2 * D], f32, tag="sel2")
            nc.tensor.transpose(cnt_ps[:C, :1], cnt_row, one_f32[:1, :1])
            # ---- assemble [delta | counts] and reduce across cores ----
            cat = finpool.tile([P, CC_W], f32)
            nc.vector.memset(cat[:, :], 0.0)
            nc.scalar.copy(cat[:C, :D], psum_delta[:C, :])
            nc.scalar.copy(cat[:C, D:D + 1], cnt_ps[:C, :1])
            if with_cc:
                cc_in = drampool.tile([C_PAD, CC_W], f32)
                cc_out = drampool.tile([C_SH, CC_W], f32)
                nc.sync.dma_start(cc_in, cat[:C_PAD, :])
                nc.gpsimd.collective_compute(
                    "ReduceScatter", mybir.AluOpType.add,
                    replica_groups=[list(range(n_cores))],
                    ins=[cc_in.opt()], outs=[cc_out.opt()],
                )
                tot = finpool.tile([C_SH, CC_W], f32)
                nc.sync.dma_start(tot, cc_out)
            else:
                tot = cat[:C_SH, :]

            # ---- new_centers_shard = censh - ALPHA*delta/(counts+1) ----
            cntp1 = finpool.tile([C_SH, 1], f32)
            nc.scalar.add(cntp1, tot[:, D:D + 1], 1.0)
            recip = finpool.tile([C_SH, 1], f32)
            nc.vector.reciprocal(recip, cntp1)
            scaled = finpool.tile([C_SH, D], f32)
            nc.vector.tensor_scalar(out=scaled, in0=tot[:, :D],
                                    scalar1=recip, scalar2=ALPHA,
                                    op0=mult, op1=mult)
            ncen_sb = finpool.tile([C_SH, D], f32)
            nc.vector.tensor_tensor(out=ncen_sb, in0=censh_sb,
                                    in1=scaled, op=sub)

            nc.sync.dma_start(ncen_d, ncen_sb)
            nc.sync.dma_start(res_d, res_sb)

    nc.compile()
    return nc


def _get_nc():
    global _NC_CACHE
    if _NC_CACHE is None:
        _NC_CACHE = _build()
    return _NC_CACHE


def _shard_inputs(feats, onehot, centers, n_cores=N_CORES, b_local=B_LOCAL):
    cen_pad = np.zeros((C_PAD, D), np.float32)
    cen_pad[:C] = centers
    in_maps = []
    for k in range(n_cores):
        sl = slice(k * b_local, (k + 1) * b_local)
        in_maps.append({
            "features": feats[sl],
            "onehot": onehot[sl],
            "centers": centers,
            "centers_shard": np.ascontiguousarray(
                cen_pad[k * C_SH:(k + 1) * C_SH]),
        })
    return in_maps


def _run(inputs, trace=False, **run_kwargs):
    from concourse.bass_utils import run_bass_kernel_spmd

    feats = np.ascontiguousarray(np.asarray(inputs["features"],
                                            dtype=np.float32))
    onehot = np.ascontiguousarray(np.asarray(inputs["onehot"],
                                             dtype=np.float32))
    centers = np.ascontiguousarray(np.asarray(inputs["centers"],
                                              dtype=np.float32))
    assert feats.shape == (B, D) and onehot.shape == (B, C)
    assert centers.shape == (C, D)

    nc = _get_nc()
    in_maps = _shard_inputs(feats, onehot, centers)
    bres = run_bass_kernel_spmd(nc, in_maps, core_ids=list(range(N_CORES)),
                                trace=trace, **run_kwargs)
    outs = bres.results
    result = np.concatenate(
        [outs[k]["result_pt"].T.reshape(-1, 1) for k in range(N_CORES)],
        axis=0,
    ).astype(np.float32)
    new_centers = np.concatenate(
        [outs[k]["new_centers_shard"] for k in range(N_CORES)], axis=0,
    )[:C].astype(np.float32)
    return (result, new_centers), bres


def kernel(**inputs):
    (result, new_centers), _ = _run(inputs, trace=False)
    return result, new_centers
